# revision 1
# baseline (speedup 1.0000x reference)
"""GAT (3-layer, PyG GATConv-style) Trainium2 Bass kernel, 8-core SPMD.

Strategy (dst-sharded edge parallelism):
  - Pad N to NPAD (multiple of 1024). Core c owns node range [c*NPAD/8, (c+1)*NPAD/8),
    i.e. BPC = NPAD/1024 blocks of 128 dst nodes.
  - Host: append self-loops, sort edges by dst, assign each edge to the core that
    owns its dst, pad each (core, block) to T tiles of 128 edges. Indices/slots are
    shipped as per-core int16/bf16 tables; the device program is identical on all
    cores (same NEFF), only input data differs.
  - Per layer L: each core computes h_aug = x @ W_aug for its own nodes
    (W_aug has fused per-head attention projections a_src/a_dst as extra columns),
    stores rows [h | s_src | s_dst | pad] to DRAM, AllGather -> full table.
    Then per dst block: dma_gather rows by src (features+s_src) and by dst
    (s_dst slice); p = exp(leakyrelu(s_src+s_dst)); one-hot (edge->dst slot)
    matmuls aggregate numer = sum p*h and denom = sum p into PSUM; normalize,
    mean heads, +bias, relu -> next layer input (kept transposed in SBUF).
  - Layer 3 ends with a ones-vector matmul accumulating the node-mean partial;
    host sums the 8 per-core [1,128] partials.
"""

import numpy as np
import ml_dtypes

BF16 = ml_dtypes.bfloat16
NCORES = 8


# ----------------------------------------------------------------------------
# Host-side preprocessing
# ----------------------------------------------------------------------------

def _wrap16(idx_flat):
    """dma_gather index layout: [128, n/16] int16, idx i at [i%16, i//16],
    replicated across the 8 groups of 16 partitions."""
    n = idx_flat.shape[0]
    assert n % 16 == 0
    w = idx_flat.reshape(n // 16, 16).T.astype(np.int16)  # [16, n/16]
    return np.tile(w, (8, 1))  # [128, n/16]


def prep_static(edge_index, N, NPAD):
    """Edge structure -> per-core gather/slot tables. Returns (T, idxs, idxd, slot)."""
    E0 = edge_index.shape[1]
    loops = np.arange(N, dtype=np.int64)
    src = np.concatenate([edge_index[0].astype(np.int64), loops])
    dst = np.concatenate([edge_index[1].astype(np.int64), loops])
    order = np.argsort(dst, kind="stable")
    src_s, dst_s = src[order], dst[order]

    BPC = NPAD // (128 * NCORES)
    n_blocks = NPAD // 128
    NPC = NPAD // NCORES
    CR = 512 if NPC % 512 == 0 else NPC  # allgather chunk rows (<1MB/rank -> mesh)
    # hf row layout after chunked allgather: chunk k holds rank-c rows
    # [k*CR,(k+1)*CR) at hf rows k*CR*8 + c*CR + r%CR
    def node2row(n):
        c, r = n // NPC, n % NPC
        return (r // CR) * (CR * NCORES) + c * CR + (r % CR)
    # contiguous edge range per global block
    bounds = np.searchsorted(dst_s, np.arange(n_blocks + 1) * 128)
    counts = bounds[1:] - bounds[:-1]
    # per-block-index tile count: max over cores for that block position
    counts_cb = counts.reshape(NCORES, BPC)
    TBS = tuple(int(t) for t in np.maximum(
        1, np.ceil(counts_cb.max(axis=0) / 128).astype(np.int64)))

    idxs_cores, idxd_cores, slot_cores = [], [], []
    for c in range(NCORES):
        iw_cols, dw_cols, sl_cols = [], [], []
        for b in range(BPC):
            Tb = TBS[b]
            g = c * BPC + b
            lo, hi = int(bounds[g]), int(bounds[g + 1])
            n_e = hi - lo
            gsrc = np.zeros(Tb * 128, dtype=np.int64)
            gdst = np.zeros(Tb * 128, dtype=np.int64)
            gslot = np.full(Tb * 128, 255.0, dtype=np.float32)
            gsrc[:n_e] = node2row(src_s[lo:hi])
            gdst[:n_e] = node2row(dst_s[lo:hi])
            gslot[:n_e] = (dst_s[lo:hi] - g * 128).astype(np.float32)
            iw_cols.append(_wrap16(gsrc))
            dw_cols.append(_wrap16(gdst))
            # edge i of block -> (tile t=i//128, partition p=i%128)
            sl_cols.append(gslot.reshape(Tb, 128).T)  # [128, Tb]
        idxs_cores.append(np.concatenate(iw_cols, axis=1))
        idxd_cores.append(np.concatenate(dw_cols, axis=1))
        slot_cores.append(np.concatenate(sl_cols, axis=1).astype(np.float32))
    oh_cores = []
    for c in range(NCORES):
        sl = slot_cores[c]  # [128, sum(TBS)] float32
        oh = (sl[:, :, None] == np.arange(128, dtype=np.float32)[None, None, :])
        oh_cores.append(np.ascontiguousarray(
            oh.astype(BF16).reshape(128, -1)))  # [128, BPC*T*128]
    return TBS, idxs_cores, idxd_cores, oh_cores, CR


def prep_values(x, Ws, a_srcs, a_dsts, bs, NPAD):
    """Cast/fuse parameters. Returns dict of host arrays shared by all cores
    (except xT which is per-core sliced by the caller)."""
    N, F = x.shape
    xp = np.zeros((NPAD, F), dtype=np.float32)
    xp[:N] = x
    xT = np.ascontiguousarray(xp.T).astype(BF16)  # [F, NPAD]

    W_augs = []
    for W, a_s, a_d in zip(Ws, a_srcs, a_dsts):
        H, Fin, C = W.shape
        RW = _row_width(H, C)
        Wf = np.transpose(W, (1, 0, 2)).reshape(Fin, H * C)
        wsrc = np.einsum("hfc,hc->fh", W, a_s)
        wdst = np.einsum("hfc,hc->fh", W, a_d)
        off = H * C + (1 if H == 1 else 0)  # H==1: col H*C is the ones col
        Wa = np.zeros((Fin, RW), dtype=np.float32)
        Wa[:, : H * C] = Wf
        Wa[:, off : off + H] = wsrc
        Wa[:, off + H : off + 2 * H] = wdst
        W_augs.append(Wa.astype(BF16))
    return xT, W_augs


def _row_width(H, C):
    """h_aug row width (elements): H*C features + 2H scores, padded so the
    bf16 row is a multiple of 256 bytes (=128 elements)."""
    used = H * C + 2 * H
    return ((used + 127) // 128) * 128


# ----------------------------------------------------------------------------
# Device program
# ----------------------------------------------------------------------------

def build_nc(cfg, repeat=1):
    import concourse.bacc as bacc
    import concourse.bass as bass
    import concourse.mybir as mybir
    import concourse.tile as tile
    from concourse.masks import make_identity
    from contextlib import ExitStack

    f32 = mybir.dt.float32
    bf16 = mybir.dt.bfloat16
    i16 = mybir.dt.int16
    ALU = mybir.AluOpType
    ACT = mybir.ActivationFunctionType

    N = cfg["N"]
    NPAD = cfg["NPAD"]
    F_IN = cfg["F_IN"]
    C = cfg["C"]
    TBS = cfg["TBS"]            # tiles per block index
    SUMT = sum(TBS)
    tb_off = [0]
    for tb in TBS:
        tb_off.append(tb_off[-1] + tb)
    HS = cfg["HS"]              # heads per layer, e.g. (4, 4, 1)
    BPC = NPAD // (128 * NCORES)
    CR = cfg["CR"]
    NPC = NPAD // NCORES
    NCH = NPC // CR
    NL = len(HS)
    RWs = [_row_width(H, C) for H in HS]
    FINs = [F_IN] + [C] * (NL - 1)

    nc = bacc.Bacc("TRN2", target_bir_lowering=False, debug=False,
                   num_devices=NCORES)

    # ---- I/O ----
    xT_d = nc.dram_tensor("xT", [F_IN, NPAD // NCORES], bf16, kind="ExternalInput")
    idxs_d = nc.dram_tensor("idxs", [128, SUMT * 8], i16, kind="ExternalInput")
    idxd_d = nc.dram_tensor("idxd", [128, SUMT * 8], i16, kind="ExternalInput")
    oh_d = nc.dram_tensor("oh", [128, SUMT * 128], bf16, kind="ExternalInput")
    W_d = [nc.dram_tensor(f"w{i+1}", [FINs[i], RWs[i]], bf16, kind="ExternalInput")
           for i in range(NL)]
    bb_d = [nc.dram_tensor(f"bb{i+1}", [C, 1], f32, kind="ExternalInput")
            for i in range(NL - 1)]
    b3r_d = nc.dram_tensor("b3r", [1, C], f32, kind="ExternalInput")
    out_d = nc.dram_tensor("out", [1, C], f32, kind="ExternalOutput")

    with tile.TileContext(nc, num_cores=NCORES) as tc, ExitStack() as ctx:
        dram = ctx.enter_context(tc.tile_pool(name="dram", bufs=1, space="DRAM"))
        cpool = ctx.enter_context(tc.tile_pool(name="consts", bufs=1))
        gpool = ctx.enter_context(tc.tile_pool(name="gath", bufs=3))
        ohpool = ctx.enter_context(tc.tile_pool(name="oh", bufs=4))
        wpool = ctx.enter_context(tc.tile_pool(name="work", bufs=3))
        fpool = ctx.enter_context(tc.tile_pool(name="fin", bufs=2))
        hpool = ctx.enter_context(tc.tile_pool(name="haug", bufs=3))
        psum = ctx.enter_context(tc.tile_pool(name="ps", bufs=2, space="PSUM"))

        # DRAM scratch (pool tiles so Tile tracks collective/gather deps)
        hl = [dram.tile([NPAD // NCORES, RWs[i]], bf16, tag=f"hl{i}",
                        name=f"hl{i}") for i in range(NL)]
        hf = [dram.tile([NPAD, RWs[i]], bf16, tag=f"hf{i}", name=f"hf{i}")
              for i in range(NL)]

        # ---- constants into SBUF ----
        ident = cpool.tile([128, 128], f32, tag="ident")
        make_identity(nc, ident[:])
        xT_sb = cpool.tile([F_IN, NPAD // NCORES], bf16, tag="xT")
        nc.sync.dma_start(xT_sb[:], xT_d[:, :])
        idxs_sb = cpool.tile([128, SUMT * 8], i16, tag="idxs")
        nc.sync.dma_start(idxs_sb[:], idxs_d[:, :])
        idxd_sb = cpool.tile([128, SUMT * 8], i16, tag="idxd")
        nc.sync.dma_start(idxd_sb[:], idxd_d[:, :])
        W_sb = []
        for i in range(NL):
            w = cpool.tile([FINs[i], RWs[i]], bf16, tag=f"w{i}", name=f"w{i}")
            nc.sync.dma_start(w[:], W_d[i][:, :])
            W_sb.append(w)
        bb_sb = []
        for i in range(NL - 1):
            b = cpool.tile([C, 1], f32, tag=f"bb{i}", name=f"bb{i}")
            nc.sync.dma_start(b[:], bb_d[i][:, :])
            bb_sb.append(b)
        b3_sb = cpool.tile([1, C], f32, tag="b3")
        nc.sync.dma_start(b3_sb[:], b3r_d[:, :])
        ones_sb = cpool.tile([128, 1], f32, tag="ones")
        nc.vector.memset(ones_sb[:], 1.0)

        # next-layer transposed features, per layer boundary
        x2T = [cpool.tile([128, NPAD // NCORES], bf16, tag=f"x2T{i}",
                          name=f"x2T{i}") for i in range(NL - 1)]

        pfin = psum.tile([1, C], f32, tag="pfin", bufs=1)

        for _rep in range(repeat):
         for L in range(NL):
             H = HS[L]
             RW = RWs[L]
             SOFF = H * C + (1 if H == 1 else 0)  # s_src offset (H==1: skip ones col)
             S2 = 128                          # gather2 slice width (256B)
             s2off = (SOFF // 128) * 128       # aligned slice start covering s cols
             s_src_in2 = SOFF - s2off          # s_src position inside slice
             HC = H * C

             # ---- phase A: h_aug for own nodes ----
             for b in range(BPC):
                 if L == 0:
                     lhs = xT_sb[:, b * 128:(b + 1) * 128]
                 else:
                     lhs = x2T[L - 1][:, b * 128:(b + 1) * 128]
                 hs = hpool.tile([128, RW], bf16, tag="hs")
                 if RW > 512:
                     p1 = psum.tile([128, 512], f32, tag="pnum")
                     nc.tensor.matmul(p1[:], lhs, W_sb[L][:, 0:512],
                                      start=True, stop=True)
                     p2 = psum.tile([128, RW - 512], f32, tag="p128")
                     nc.tensor.matmul(p2[:], lhs, W_sb[L][:, 512:RW],
                                      start=True, stop=True)
                     nc.scalar.copy(hs[:, 0:512], p1[:])
                     nc.vector.tensor_copy(hs[:, 512:RW], p2[:])
                 else:
                     p1 = psum.tile([128, RW], f32, tag="pnum")
                     nc.tensor.matmul(p1[:], lhs, W_sb[L][:, 0:RW],
                                      start=True, stop=True)
                     nc.scalar.copy(hs[:, 0:RW], p1[:])
                 if H == 1:
                     nc.vector.memset(hs[:, HC:HC + 1], 1.0)
                 nc.sync.dma_start(hl[L][b * 128:(b + 1) * 128, :], hs[:])

             # ---- phase B: allgather, chunked <1MB/rank to stay on mesh algo ----
             for k in range(NCH):
                 nc.gpsimd.collective_compute(
                     "AllGather", mybir.AluOpType.bypass,
                     replica_groups=[list(range(NCORES))],
                     ins=[hl[L][k * CR:(k + 1) * CR, :].opt()],
                     outs=[hf[L][k * CR * NCORES:(k + 1) * CR * NCORES, :].opt()],
                 )

             # ---- phase C: edge aggregation per dst block ----
             GC = 6  # tiles per gather chunk (768 idxs = 48 desc/engine <= 64-desc packet limit)
             for b in range(BPC):
                 T = TBS[b]
                 base = tb_off[b]
                 chunks = [(c0, min(GC, T - c0)) for c0 in range(0, T, GC)]
                 NW = HC + 1 if H == 1 else HC  # H==1: denom rides as col C
                 numer = psum.tile([128, NW], f32, tag="pnum")
                 if H > 1:
                     denom = psum.tile([128, H], f32, tag="pden")
                 g1s, g2s = [], []
                 sc = wpool.tile([128, T, H], f32, tag="sc")
                 ohc = ohpool.tile([128, T * 128], bf16, tag="ohc", bufs=2)
                 nc.sync.dma_start(
                     ohc[:], oh_d[:, base * 128:(base + T) * 128])
                 for c0, tc_n in chunks:
                     ic = slice((base + c0) * 8, (base + c0 + tc_n) * 8)
                     g1 = gpool.tile([128, tc_n, RW], bf16, tag="g1", bufs=8)
                     nc.gpsimd.dma_gather(g1[:], hf[L][:, :], idxs_sb[:, ic],
                                          tc_n * 128, tc_n * 128, RW)
                     g2 = gpool.tile([128, tc_n, S2], bf16, tag="g2", bufs=8)
                     nc.gpsimd.dma_gather(g2[:], hf[L][:, s2off:s2off + S2],
                                          idxd_sb[:, ic], tc_n * 128, tc_n * 128,
                                          S2, elem_step=RW)
                     nc.vector.tensor_tensor(
                         sc[:, c0:c0 + tc_n, :], g1[:, :, SOFF:SOFF + H],
                         g2[:, :, s_src_in2 + H:s_src_in2 + 2 * H], ALU.add)
                     g1s.append(g1); g2s.append(g2)

                 # p = exp(leakyrelu(sc)), batched per block [128, T, H]
                 lr = wpool.tile([128, T, H], f32, tag="lr")
                 nc.vector.tensor_scalar(lr[:], sc[:], 0.2, None, op0=ALU.mult)
                 lr2 = wpool.tile([128, T, H], f32, tag="lr2")
                 nc.vector.tensor_tensor(lr2[:], lr[:], sc[:], ALU.max)
                 p = wpool.tile([128, T, H], f32, tag="p")
                 nc.scalar.activation(p[:], lr2[:], ACT.Exp)
                 if H > 1:
                     pb = wpool.tile([128, T, H], bf16, tag="pb")
                     nc.vector.tensor_copy(pb[:], p[:])

                 for ci, (c0, tc_n) in enumerate(chunks):
                     g1 = g1s[ci]
                     for tt in range(tc_n):
                         t = c0 + tt
                         oh_ap = ohc[:, t * 128:(t + 1) * 128]
                         msg = wpool.tile([128, NW], bf16, tag="msg")
                         if H == 1:
                             # one mul over [h | ones] -> [p*h | p]; one matmul
                             nc.vector.tensor_scalar(
                                 msg[:], g1[:, tt, 0:NW], p[:, t, 0:1],
                                 None, op0=ALU.mult)
                         else:
                             for h in range(H):
                                 src_ap = g1[:, tt, h * C:(h + 1) * C]
                                 dst_ap = msg[:, h * C:(h + 1) * C]
                                 pcol = p[:, t, h:h + 1]
                                 if h % 2 == 0:
                                     nc.vector.tensor_scalar(dst_ap, src_ap,
                                                             pcol, None,
                                                             op0=ALU.mult)
                                 else:
                                     nc.scalar.mul(dst_ap, src_ap, pcol)
                         nc.tensor.matmul(numer[:], oh_ap, msg[:],
                                          start=(t == 0), stop=(t == T - 1))
                         if H > 1:
                             nc.tensor.matmul(denom[:], oh_ap, pb[:, t, :],
                                              start=(t == 0), stop=(t == T - 1))

                 # ---- finalize block ----
                 dn = fpool.tile([128, H], f32, tag="dn")
                 dsrc = denom[:] if H > 1 else numer[:, HC:HC + 1]
                 nc.vector.tensor_scalar(dn[:], dsrc, float(H), 1e-16 * H,
                                         op0=ALU.mult, op1=ALU.add)
                 rc = fpool.tile([128, H], f32, tag="rc")
                 nc.vector.reciprocal(rc[:], dn[:])
                 if L < NL - 1:
                     ms = []
                     for h in range(H):
                         m = fpool.tile([128, C], f32, tag=f"m{h}", name=f"m{h}")
                         if h % 2 == 0:
                             nc.vector.tensor_scalar(
                                 m[:], numer[:, h * C:(h + 1) * C],
                                 rc[:, h:h + 1], None, op0=ALU.mult)
                         else:
                             nc.scalar.mul(m[:], numer[:, h * C:(h + 1) * C],
                                           rc[:, h:h + 1])
                         ms.append(m)
                     acc = ms[0]
                     if H > 1:
                         s01 = fpool.tile([128, C], f32, tag="s01")
                         nc.vector.tensor_tensor(s01[:], ms[0][:], ms[1][:], ALU.add)
                         acc = s01
                         if H == 4:
                             s23 = fpool.tile([128, C], f32, tag="s23")
                             nc.vector.tensor_tensor(s23[:], ms[2][:], ms[3][:],
                                                     ALU.add)
                             s4 = fpool.tile([128, C], f32, tag="s4")
                             nc.vector.tensor_tensor(s4[:], s01[:], s23[:], ALU.add)
                             acc = s4
                     pt = psum.tile([128, 128], f32, tag="p128")
                     nc.tensor.transpose(pt[:], acc[:], ident[:])
                     nc.scalar.activation(x2T[L][:, b * 128:(b + 1) * 128],
                                          pt[:], ACT.Relu, bias=bb_sb[L][:])
                 else:
                     o3 = fpool.tile([128, C], f32, tag="o3")
                     nc.vector.tensor_scalar(o3[:], numer[:, 0:C], rc[:, 0:1],
                                             None, op0=ALU.mult)
                     nc.tensor.matmul(pfin[:], ones_sb[:], o3[:],
                                      start=(b == 0), stop=(b == BPC - 1))

        fs = fpool.tile([1, C], f32, tag="fs")
        nc.vector.tensor_scalar(fs[:], pfin[:], 1.0 / N, None, op0=ALU.mult)
        fs2 = fpool.tile([1, C], f32, tag="fs2")
        nc.vector.tensor_tensor(fs2[:], fs[:], b3_sb[:], ALU.add)
        nc.sync.dma_start(out_d[:, :], fs2[:])

    nc.compile()
    return nc


# ----------------------------------------------------------------------------
# Entry points
# ----------------------------------------------------------------------------

def make_cfg_and_maps(inputs):
    x = np.asarray(inputs["x"])
    edge_index = np.asarray(inputs["edge_index"])
    N, F_IN = x.shape
    NPAD = ((N + 1023) // 1024) * 1024
    Ws = [np.asarray(inputs[f"W{i}"]) for i in (1, 2, 3)]
    a_srcs = [np.asarray(inputs[f"as{i}"]) for i in (1, 2, 3)]
    a_dsts = [np.asarray(inputs[f"ad{i}"]) for i in (1, 2, 3)]
    bs = [np.asarray(inputs[f"b{i}"]) for i in (1, 2, 3)]
    HS = tuple(W.shape[0] for W in Ws)
    C = Ws[0].shape[2]

    TBS, idxs_c, idxd_c, oh_c, CR = prep_static(edge_index, N, NPAD)
    xT, W_augs = prep_values(x, Ws, a_srcs, a_dsts, bs, NPAD)

    cfg = dict(N=N, NPAD=NPAD, F_IN=F_IN, C=C, TBS=TBS, HS=HS, CR=CR)
    NPC = NPAD // NCORES
    in_maps = []
    for c in range(NCORES):
        m = {
            "xT": np.ascontiguousarray(xT[:, c * NPC:(c + 1) * NPC]),
            "idxs": idxs_c[c],
            "idxd": idxd_c[c],
            "oh": oh_c[c],
            "b3r": (bs[2] * (1.0 / NCORES)).reshape(1, C).astype(np.float32),
        }
        for i in range(3):
            m[f"w{i+1}"] = W_augs[i]
        for i in range(2):
            m[f"bb{i+1}"] = bs[i].astype(np.float32).reshape(C, 1)
        in_maps.append(m)
    return cfg, in_maps


_NC_CACHE = {}


def _get_nc(cfg, repeat=1):
    key = (repeat,) + tuple(sorted((k, v if not isinstance(v, tuple) else v)
                                   for k, v in cfg.items()))
    if key not in _NC_CACHE:
        _NC_CACHE[key] = build_nc(cfg, repeat=repeat)
    return _NC_CACHE[key]


def run(inputs, trace=False, repeat=1, **kw):
    from concourse.bass_utils import run_bass_kernel_spmd
    cfg, in_maps = make_cfg_and_maps(inputs)
    nc = _get_nc(cfg, repeat=repeat)
    res = run_bass_kernel_spmd(nc, in_maps, core_ids=list(range(NCORES)),
                               trace=trace, **kw)
    out = np.zeros((1, cfg["C"]), dtype=np.float32)
    for r in res.results:
        out += r["out"]
    return out, res


def kernel(**inputs) -> np.ndarray:
    out, _ = run(inputs)
    return out



# revision 11
# speedup vs baseline: 17.7163x; 17.7163x over previous
"""GAT (3-layer, PyG GATConv-style) Trainium2 Bass kernel, 8-core SPMD.

Instruction-count-optimized rewrite. The axon execution path costs ~constant
time per instruction, so the kernel is organized to touch as many elements as
possible per instruction:

  - Nodes are permuted by in-degree and dealt round-robin to the 8 cores, so
    each 128-dst "block" has near-uniform in-degree. Incoming edges of block b
    are packed into a dense [128 dst-slot, KB_b] grid (k-major), padded with a
    dedicated -inf row so padded slots contribute exp(-large)=0.
  - Per block, ONE dma_gather chunk pulls up to 1024 src rows (features +
    fused attention scores) into [128, kn, RW]; attention + softmax numer/
    denom are computed with ~6 wide vector ops per block using broadcast
    access patterns and free-dim tensor_reduce (no per-tile matmuls).
  - Segment softmax normalization, head-mean, bias, relu are batched across
    all 20 blocks in ~7 instructions; the next-layer transposed input is
    produced by a single DMA-transpose (XBAR).
  - Layer tables are AllGathered once per layer in a single collective.

Node bookkeeping: within a core, node t = b*128 + s (block b, slot s) is
column t of xT/x2T and row t of the local hl table; its global hf row is
core*ROWPAD + t. The XBAR DMA-transpose is a blockwise 128-column panel
transpose: out[p, j, s] = in[s, j*128 + p] for a 3D [128, B, 128] out AP.
"""

import numpy as np
import ml_dtypes

BF16 = ml_dtypes.bfloat16
NCORES = 8
PADR = 16          # -inf pad rows appended to each core's hl table
GC = 8             # k-slices per gather (8*128 = 1024 idxs)
SCW = 16           # score column block width (H src + H dst + zeros)


# ----------------------------------------------------------------------------
# Host-side preprocessing
# ----------------------------------------------------------------------------

def _wrap16(idx_flat):
    """dma_gather index layout: [128, n/16] int16, idx i at [i%16, i//16],
    replicated across the 8 groups of 16 partitions."""
    n = idx_flat.shape[0]
    assert n % 16 == 0
    w = idx_flat.reshape(n // 16, 16).T.astype(np.int16)  # [16, n/16]
    return np.tile(w, (8, 1))  # [128, n/16]


def prep_static(edge_index, N, NPAD):
    """Degree-balanced node permutation + dense per-block gather tables.

    Returns (KB, idx_cores, perm) where perm[c] lists original node ids in
    core-c slot order t=0..NPC-1 (pad slots = -1)."""
    NPC = NPAD // NCORES
    B = NPC // 128
    ROWPAD = NPC + PADR

    E0 = edge_index.shape[1]
    src0 = edge_index[0].astype(np.int64)
    dst0 = edge_index[1].astype(np.int64)
    deg = np.zeros(NPAD, dtype=np.int64)
    np.add.at(deg, dst0, 1)
    deg[:N] += 1  # self-loop
    order = np.argsort(-deg, kind="stable")      # positions -> orig node
    pos = np.empty(NPAD, dtype=np.int64)
    pos[order] = np.arange(NPAD)
    core_of = pos % NCORES
    slot_of = pos // NCORES                      # t within core (block-major)

    # per-block k-capacity: max degree among the block's 1024 sorted positions
    KB = [int(max(1, deg[order[b * 128 * NCORES]])) for b in range(B)]
    off = np.zeros(B + 1, dtype=np.int64)
    for b in range(B):
        off[b + 1] = off[b] + KB[b] * 128

    # row index within core table = slot t (block-major: t = b*128 + s)
    row_of = core_of * ROWPAD + slot_of          # global hf row per node
    PADROW = NPC                                  # core 0's first -inf row

    # edges incl self-loops, grouped per destination
    src = np.concatenate([src0, np.arange(N, dtype=np.int64)])
    dst = np.concatenate([dst0, np.arange(N, dtype=np.int64)])
    key = core_of[dst] * NPC + slot_of[dst]
    eorder = np.argsort(key, kind="stable")
    key_s = key[eorder]
    srcrow_s = row_of[src[eorder]]
    cnt = np.bincount(key_s, minlength=NCORES * NPC)
    run_start = np.zeros(NCORES * NPC, dtype=np.int64)
    run_start[1:] = np.cumsum(cnt)[:-1]
    k_e = np.arange(len(key_s)) - run_start[key_s]

    c_e = key_s // NPC
    t_e = key_s % NPC
    b_e = t_e // 128
    s_e = t_e % 128
    assert (k_e < np.array(KB)[b_e]).all()

    SUMKB = int(off[-1])
    idx_cores = []
    for c in range(NCORES):
        tab = np.full(SUMKB, PADROW, dtype=np.int64)
        m = c_e == c
        tab[off[b_e[m]] + k_e[m] * 128 + s_e[m]] = srcrow_s[m]
        idx_cores.append(_wrap16(tab))
    perm = [order[np.arange(NPC) * NCORES + c] for c in range(NCORES)]
    perm = [np.where(p < N, p, -1) for p in perm]
    return KB, idx_cores, perm


def _row_width(H, C):
    """table row width: H*C features + SCW score block, padded to 128 elems."""
    used = H * C + SCW
    return ((used + 127) // 128) * 128


def prep_values(x, Ws, a_srcs, a_dsts, perm_c, B):
    """Per-core xT (permuted to r=s*B+b column order) + fused W tables."""
    N, F = x.shape
    NPC = perm_c[0].shape[0]
    xTs = []
    for p in perm_c:
        xp = np.zeros((NPC, F), dtype=np.float32)
        ok = p >= 0
        xp[ok] = x[p[ok]]
        xTs.append(np.ascontiguousarray(xp.T).astype(BF16))  # [F, NPC]

    W_augs = []
    for W, a_s, a_d in zip(Ws, a_srcs, a_dsts):
        H, Fin, C = W.shape
        RW = _row_width(H, C)
        Wf = np.transpose(W, (1, 0, 2)).reshape(Fin, H * C)
        wsrc = np.einsum("hfc,hc->fh", W, a_s)
        wdst = np.einsum("hfc,hc->fh", W, a_d)
        Wa = np.zeros((Fin, RW), dtype=np.float32)
        Wa[:, : H * C] = Wf
        Wa[:, H * C : H * C + H] = wsrc
        Wa[:, H * C + H : H * C + 2 * H] = wdst
        W_augs.append(Wa.astype(BF16))
    return xTs, W_augs


# ----------------------------------------------------------------------------
# Device program
# ----------------------------------------------------------------------------

def build_nc(cfg, repeat=1):
    import concourse.bacc as bacc
    import concourse.mybir as mybir
    import concourse.tile as tile
    from contextlib import ExitStack

    f32 = mybir.dt.float32
    bf16 = mybir.dt.bfloat16
    i16 = mybir.dt.int16
    ALU = mybir.AluOpType
    ACT = mybir.ActivationFunctionType

    N = cfg["N"]
    NPAD = cfg["NPAD"]
    F_IN = cfg["F_IN"]
    C = cfg["C"]
    KB = cfg["KB"]
    HS = cfg["HS"]
    NPC = NPAD // NCORES
    B = NPC // 128
    ROWPAD = NPC + PADR
    NL = len(HS)
    RWs = [_row_width(H, C) for H in HS]
    FINs = [F_IN] + [C] * (NL - 1)
    SUMKB = sum(KB)
    off = [0]
    for kb in KB:
        off.append(off[-1] + kb * 128)

    nc = bacc.Bacc("TRN2", target_bir_lowering=False, debug=False,
                   num_devices=NCORES)

    # ---- I/O ----
    xT_d = nc.dram_tensor("xT", [F_IN, NPC], bf16, kind="ExternalInput")
    idx_d = nc.dram_tensor("idx", [128, SUMKB * 8], i16, kind="ExternalInput")
    W_d = [nc.dram_tensor(f"w{i+1}", [FINs[i], RWs[i]], bf16,
                          kind="ExternalInput") for i in range(NL)]
    bb_d = [nc.dram_tensor(f"bb{i+1}", [128, C], f32, kind="ExternalInput")
            for i in range(NL - 1)]
    b3r_d = nc.dram_tensor("b3r", [1, C], f32, kind="ExternalInput")
    out_d = nc.dram_tensor("out", [1, C], f32, kind="ExternalOutput")

    with tile.TileContext(nc, num_cores=NCORES) as tc, ExitStack() as ctx:
        dram = ctx.enter_context(tc.tile_pool(name="dram", bufs=1, space="DRAM"))
        cpool = ctx.enter_context(tc.tile_pool(name="consts", bufs=1))
        gpool = ctx.enter_context(tc.tile_pool(name="gath", bufs=1))
        wpool = ctx.enter_context(tc.tile_pool(name="work", bufs=1))
        psum = ctx.enter_context(tc.tile_pool(name="ps", bufs=1, space="PSUM"))

        hl = [dram.tile([ROWPAD, RWs[i]], bf16, tag=f"hl{i}", name=f"hl{i}")
              for i in range(NL)]
        hf = [dram.tile([ROWPAD * NCORES, RWs[i]], bf16, tag=f"hf{i}",
                        name=f"hf{i}") for i in range(NL)]

        # ---- constants into SBUF (outside the timed repeat loop) ----
        xT_sb = cpool.tile([F_IN, NPC], bf16, tag="xT")
        nc.sync.dma_start(xT_sb[:], xT_d[:, :])
        idx_sb = cpool.tile([128, SUMKB * 8], i16, tag="idx")
        nc.sync.dma_start(idx_sb[:], idx_d[:, :])
        W_sb = []
        for i in range(NL):
            w = cpool.tile([FINs[i], RWs[i]], bf16, tag=f"w{i}", name=f"w{i}")
            nc.sync.dma_start(w[:], W_d[i][:, :])
            W_sb.append(w)
        bb_sb = []
        for i in range(NL - 1):
            b = cpool.tile([128, C], f32, tag=f"bb{i}", name=f"bb{i}")
            nc.sync.dma_start(b[:], bb_d[i][:, :])
            bb_sb.append(b)
        b3_sb = cpool.tile([1, C], f32, tag="b3")
        nc.sync.dma_start(b3_sb[:], b3r_d[:, :])
        ones_sb = cpool.tile([128, 1], f32, tag="ones")
        nc.vector.memset(ones_sb[:], 1.0)
        # -inf pad rows (scores -> exp ~ 0; features multiplied by 0)
        ninf = cpool.tile([PADR, max(RWs)], bf16, tag="ninf")
        nc.vector.memset(ninf[:], -30000.0)
        for i in range(NL):
            nc.sync.dma_start(hl[i][NPC:ROWPAD, :], ninf[:, 0:RWs[i]])

        # next-layer transposed features
        x2T = [cpool.tile([128, NPC], bf16, tag=f"x2T{i}", name=f"x2T{i}")
               for i in range(NL - 1)]

        for _rep in range(repeat):
         for L in range(NL):
            H = HS[L]
            RW = RWs[L]
            FIN = FINs[L]
            HC = H * C
            HCF = min(HC, 512)   # feature matmul width (bank-limited)

            # ---- phase A: h_aug rows for own nodes ----
            staging = wpool.tile([128, B, RW], bf16, tag="stage")
            lhs = xT_sb if L == 0 else x2T[L - 1]

            # scores (transposed): scb[j, r] for j in [0, SCW)
            scb = wpool.tile([SCW, NPC], bf16, tag="scb")
            nchk = NPC // 512                     # 512-node score chunks
            for g0 in range(0, nchk, 3):
                gn = min(3, nchk - g0)
                psc = psum.tile([SCW, 3, 512], f32, tag="psc")
                for j in range(gn):
                    nc.tensor.matmul(
                        psc[:, j, :], W_sb[L][:, HC:HC + SCW],
                        lhs[:, (g0 + j) * 512:(g0 + j + 1) * 512],
                        start=True, stop=True)
                nc.scalar.copy(scb[:, g0 * 512:(g0 + gn) * 512],
                               psc[:, 0:gn, :])
            # scores -> staging[:, :, HC:HC+SCW] via XBAR transpose
            nc.sync.dma_start(staging[:, :, HC:HC + SCW], scb[:],
                              transpose=True)

            # features: 128-node chunks, groups of 3 PSUM banks
            for g0 in range(0, B, 3):
                gn = min(3, B - g0)
                pf = psum.tile([128, 3, 512], f32, tag="pf")
                for j in range(gn):
                    nc.tensor.matmul(
                        pf[:, j, 0:HCF], lhs[:, (g0 + j) * 128:(g0 + j + 1) * 128],
                        W_sb[L][:, 0:HCF], start=True, stop=True)
                nc.scalar.copy(staging[:, g0:g0 + gn, 0:HCF], pf[:, 0:gn, 0:HCF])
            if HC > HCF:
                for g0 in range(0, B, 3):
                    gn = min(3, B - g0)
                    pf = psum.tile([128, 3, 512], f32, tag="pf")
                    for j in range(gn):
                        nc.tensor.matmul(
                            pf[:, j, 0:HC - HCF],
                            lhs[:, (g0 + j) * 128:(g0 + j + 1) * 128],
                            W_sb[L][:, HCF:HC], start=True, stop=True)
                    nc.scalar.copy(staging[:, g0:g0 + gn, HCF:HC],
                                   pf[:, 0:gn, 0:HC - HCF])

            # staging -> hl rows r = chunk*128 + part
            nc.sync.dma_start(
                hl[L][0:NPC, :].rearrange("(ch p) w -> p ch w", p=128),
                staging[:])

            # ---- phase B: allgather ----
            nc.gpsimd.collective_compute(
                "AllGather", mybir.AluOpType.bypass,
                replica_groups=[list(range(NCORES))],
                ins=[hl[L][:, :].opt()],
                outs=[hf[L][:, :].opt()],
            )

            # s_dst for own nodes: block b slot s -> row s*B + b
            sdst = wpool.tile([128, B, H], bf16, tag="sdst")
            nc.sync.dma_start(
                sdst[:],
                hl[L][0:NPC, HC + H:HC + 2 * H].rearrange(
                    "(b s) h -> s b h", s=128))

            # accumulators over all blocks
            numer = wpool.tile([128, B, HC], f32, tag="numer")
            denom = wpool.tile([128, B, H], f32, tag="denom")

            # ---- phase C: per dst block ----
            for b in range(B):
                T = KB[b]
                g1 = gpool.tile([128, T, RW], bf16, tag="g1")
                for k0 in range(0, T, GC):
                    kn = min(GC, T - k0)
                    ic = slice((off[b] + k0 * 128) // 16,
                               (off[b] + (k0 + kn) * 128) // 16)
                    nc.gpsimd.dma_gather(g1[:, k0:k0 + kn, :], hf[L][:, :],
                                         idx_sb[:, ic], kn * 128, kn * 128,
                                         RW, elem_step=RW)
                sc = wpool.tile([128, T, H], f32, tag="sc")
                nc.vector.tensor_tensor(
                    sc[:], g1[:, :, HC:HC + H],
                    sdst[:, b:b + 1, :].broadcast_to([128, T, H]), ALU.add)
                lr = wpool.tile([128, T, H], f32, tag="lr")
                nc.vector.scalar_tensor_tensor(lr[:], sc[:], 0.2, sc[:],
                                               ALU.mult, ALU.max)
                p = wpool.tile([128, T, H], f32, tag="p")
                nc.scalar.activation(p[:], lr[:], ACT.Exp)
                # msg = h_src * p, in place over gathered features
                gfeat = g1[:, :, 0:HC].rearrange("q t (h c) -> q t h c", h=H)
                nc.vector.tensor_tensor(
                    gfeat, gfeat,
                    p[:].unsqueeze(3).broadcast_to([128, T, H, C]), ALU.mult)
                nc.vector.tensor_reduce(
                    numer[:, b, :], g1[:, :, 0:HC].transpose([0, 2, 1]),
                    mybir.AxisListType.X, ALU.add)
                nc.vector.tensor_reduce(
                    denom[:, b, :], p[:].transpose([0, 2, 1]),
                    mybir.AxisListType.X, ALU.add)

            # ---- finalize (batched across blocks) ----
            nc.vector.tensor_scalar(denom[:], denom[:], 1e-16 * H, None,
                                    op0=ALU.add)
            rc = wpool.tile([128, B, H], f32, tag="rc")
            nc.vector.reciprocal(rc[:], denom[:])
            nview = numer[:].rearrange("q b (h c) -> q b h c", h=H)
            nc.vector.tensor_tensor(
                nview, nview,
                rc[:].unsqueeze(3).broadcast_to([128, B, H, C]), ALU.mult)
            if L < NL - 1:
                hm = wpool.tile([128, B, C], f32, tag="hm")
                nc.vector.tensor_reduce(
                    hm[:], nview.transpose([0, 1, 3, 2]),
                    mybir.AxisListType.X, ALU.add)
                m2 = wpool.tile([128, B, C], bf16, tag="m2")
                nc.vector.scalar_tensor_tensor(
                    m2[:], hm[:], 1.0 / H,
                    bb_sb[L][:].unsqueeze(1).broadcast_to([128, B, C]),
                    ALU.mult, ALU.add)
                nc.vector.tensor_scalar(m2[:], m2[:], 0.0, None, op0=ALU.max)
                nc.sync.dma_start(
                    x2T[L][:].rearrange("q (b s) -> q b s", b=B),
                    m2[:], transpose=True)
            else:
                nsum = wpool.tile([128, C], f32, tag="nsum")
                nc.vector.tensor_reduce(
                    nsum[:], numer[:].transpose([0, 2, 1]),
                    mybir.AxisListType.X, ALU.add)
                pfin = psum.tile([1, C], f32, tag="pfin")
                nc.tensor.matmul(pfin[:], ones_sb[:], nsum[:],
                                 start=True, stop=True)
                fs = wpool.tile([1, C], f32, tag="fs")
                nc.vector.scalar_tensor_tensor(fs[:], pfin[:], 1.0 / N,
                                               b3_sb[:], ALU.mult, ALU.add)
                nc.sync.dma_start(out_d[:, :], fs[:])

    nc.compile()
    return nc


# ----------------------------------------------------------------------------
# Entry points
# ----------------------------------------------------------------------------

def make_cfg_and_maps(inputs):
    x = np.asarray(inputs["x"])
    edge_index = np.asarray(inputs["edge_index"])
    N, F_IN = x.shape
    NPAD = ((N + 1023) // 1024) * 1024
    Ws = [np.asarray(inputs[f"W{i}"]) for i in (1, 2, 3)]
    a_srcs = [np.asarray(inputs[f"as{i}"]) for i in (1, 2, 3)]
    a_dsts = [np.asarray(inputs[f"ad{i}"]) for i in (1, 2, 3)]
    bs = [np.asarray(inputs[f"b{i}"]) for i in (1, 2, 3)]
    HS = tuple(W.shape[0] for W in Ws)
    C = Ws[0].shape[2]
    B = NPAD // NCORES // 128

    KB, idx_cores, perm = prep_static(edge_index, N, NPAD)
    xTs, W_augs = prep_values(x, Ws, a_srcs, a_dsts, perm, B)

    cfg = dict(N=N, NPAD=NPAD, F_IN=F_IN, C=C, KB=tuple(KB), HS=HS)
    in_maps = []
    for c in range(NCORES):
        m = {
            "xT": xTs[c],
            "idx": idx_cores[c],
            "b3r": (bs[2] * (1.0 / NCORES)).reshape(1, C).astype(np.float32),
        }
        for i in range(3):
            m[f"w{i+1}"] = W_augs[i]
        for i in range(2):
            m[f"bb{i+1}"] = np.broadcast_to(
                bs[i].astype(np.float32), (128, C)).copy()
        in_maps.append(m)
    return cfg, in_maps


_NC_CACHE = {}


def _get_nc(cfg, repeat=1):
    key = (repeat,) + tuple(sorted((k, v if not isinstance(v, tuple) else v)
                                   for k, v in cfg.items()))
    if key not in _NC_CACHE:
        _NC_CACHE[key] = build_nc(cfg, repeat=repeat)
    return _NC_CACHE[key]


def run(inputs, trace=False, repeat=1, **kw):
    from concourse.bass_utils import run_bass_kernel_spmd
    cfg, in_maps = make_cfg_and_maps(inputs)
    nc = _get_nc(cfg, repeat=repeat)
    res = run_bass_kernel_spmd(nc, in_maps, core_ids=list(range(NCORES)),
                               trace=trace, **kw)
    out = np.zeros((1, cfg["C"]), dtype=np.float32)
    for r in res.results:
        out += r["out"]
    return out, res


def kernel(**inputs) -> np.ndarray:
    out, _ = run(inputs)
    return out


# revision 16
# speedup vs baseline: 35.1684x; 1.9851x over previous
"""GAT (3-layer, PyG GATConv-style) Trainium2 Bass kernel, 8-core SPMD.

Instruction-count-optimized rewrite. The axon execution path costs ~constant
time per instruction, so the kernel is organized to touch as many elements as
possible per instruction:

  - Nodes are permuted by in-degree and dealt round-robin to the 8 cores, so
    each 128-dst "block" has near-uniform in-degree. Incoming edges of block b
    are packed into a dense [128 dst-slot, KB_b] grid (k-major), padded with a
    dedicated -inf row so padded slots contribute exp(-large)=0.
  - Per block, ONE dma_gather chunk pulls up to 1024 src rows (features +
    fused attention scores) into [128, kn, RW]; attention + softmax numer/
    denom are computed with ~6 wide vector ops per block using broadcast
    access patterns and free-dim tensor_reduce (no per-tile matmuls).
  - Segment softmax normalization, head-mean, bias, relu are batched across
    all 20 blocks in ~7 instructions; the next-layer transposed input is
    produced by a single DMA-transpose (XBAR).
  - Layer tables are AllGathered once per layer in a single collective.

Node bookkeeping: within a core, node t = b*128 + s (block b, slot s) is
column t of xT/x2T and row t of the local hl table; its global hf row is
core*ROWPAD + t. The XBAR DMA-transpose is a blockwise 128-column panel
transpose: out[p, j, s] = in[s, j*128 + p] for a 3D [128, B, 128] out AP.
"""

import numpy as np
import ml_dtypes

BF16 = ml_dtypes.bfloat16
NCORES = 8
PADR = 16          # -inf pad rows appended to each core's hl table
GC = 8             # k-slices per gather (8*128 = 1024 idxs)
SCW = 16           # score column block width (H src + H dst + zeros)
PAIR_CAP = 72      # max 128-slices in a paired gather tile (SBUF budget)


# ----------------------------------------------------------------------------
# Host-side preprocessing
# ----------------------------------------------------------------------------

def _wrap16(idx_flat):
    """dma_gather index layout: [128, n/16] int16, idx i at [i%16, i//16],
    replicated across the 8 groups of 16 partitions."""
    n = idx_flat.shape[0]
    assert n % 16 == 0
    w = idx_flat.reshape(n // 16, 16).T.astype(np.int16)  # [16, n/16]
    return np.tile(w, (8, 1))  # [128, n/16]


def prep_static(edge_index, N, NPAD):
    """Degree-balanced node permutation + dense per-block gather tables.

    Returns (KB, idx_cores, perm) where perm[c] lists original node ids in
    core-c slot order t=0..NPC-1 (pad slots = -1)."""
    NPC = NPAD // NCORES
    B = NPC // 128
    ROWPAD = NPC + PADR

    E0 = edge_index.shape[1]
    src0 = edge_index[0].astype(np.int64)
    dst0 = edge_index[1].astype(np.int64)
    deg = np.zeros(NPAD, dtype=np.int64)
    np.add.at(deg, dst0, 1)
    deg[:N] += 1  # self-loop
    order = np.argsort(-deg, kind="stable")      # positions -> orig node
    pos = np.empty(NPAD, dtype=np.int64)
    pos[order] = np.arange(NPAD)
    core_of = pos % NCORES
    slot_of = pos // NCORES                      # t within core (block-major)

    # per-block k-capacity: max degree among the block's 1024 sorted positions
    KB = [int(max(1, deg[order[b * 128 * NCORES]])) for b in range(B)]
    # group adjacent blocks into pairs where the paired gather tile stays small
    groups = []   # (b0, gsz, KBG)
    b = 0
    while b < B:
        if b + 1 < B and 2 * max(KB[b], KB[b + 1]) <= PAIR_CAP:
            groups.append((b, 2, max(KB[b], KB[b + 1])))
            b += 2
        else:
            groups.append((b, 1, KB[b]))
            b += 1
    # per-block placement within its group's k-major interleaved table
    goffb = np.zeros(B, dtype=np.int64)
    gszb = np.zeros(B, dtype=np.int64)
    jb = np.zeros(B, dtype=np.int64)
    goff = 0
    for (b0, gsz, KBG) in groups:
        for j in range(gsz):
            goffb[b0 + j] = goff
            gszb[b0 + j] = gsz
            jb[b0 + j] = j
        goff += KBG * gsz * 128
    SUMSL = goff // 128   # total 128-slices

    # row index within core table = slot t (block-major: t = b*128 + s)
    row_of = core_of * ROWPAD + slot_of          # global hf row per node
    PADROW = NPC                                  # core 0's first -inf row

    # edges incl self-loops, grouped per destination
    src = np.concatenate([src0, np.arange(N, dtype=np.int64)])
    dst = np.concatenate([dst0, np.arange(N, dtype=np.int64)])
    key = core_of[dst] * NPC + slot_of[dst]
    eorder = np.argsort(key, kind="stable")
    key_s = key[eorder]
    srcrow_s = row_of[src[eorder]]
    cnt = np.bincount(key_s, minlength=NCORES * NPC)
    run_start = np.zeros(NCORES * NPC, dtype=np.int64)
    run_start[1:] = np.cumsum(cnt)[:-1]
    k_e = np.arange(len(key_s)) - run_start[key_s]

    c_e = key_s // NPC
    t_e = key_s % NPC
    b_e = t_e // 128
    s_e = t_e % 128
    assert (k_e < np.array(KB)[b_e]).all()

    idx_cores = []
    for c in range(NCORES):
        tab = np.full(goff, PADROW, dtype=np.int64)
        m = c_e == c
        pos = (goffb[b_e[m]] + (k_e[m] * gszb[b_e[m]] + jb[b_e[m]]) * 128
               + s_e[m])
        tab[pos] = srcrow_s[m]
        idx_cores.append(_wrap16(tab))
    perm = [order[np.arange(NPC) * NCORES + c] for c in range(NCORES)]
    perm = [np.where(p < N, p, -1) for p in perm]
    return tuple(groups), SUMSL, idx_cores, perm


def _row_width(H, C):
    """table row width: H*C features + SCW score block, padded to 128 elems."""
    used = H * C + SCW
    return ((used + 127) // 128) * 128


def prep_values(x, Ws, a_srcs, a_dsts, perm_c, B):
    """Per-core xT (permuted to r=s*B+b column order) + fused W tables."""
    N, F = x.shape
    NPC = perm_c[0].shape[0]
    xTs = []
    for p in perm_c:
        xp = np.zeros((NPC, F), dtype=np.float32)
        ok = p >= 0
        xp[ok] = x[p[ok]]
        xTs.append(np.ascontiguousarray(xp.T).astype(BF16))  # [F, NPC]

    W_augs = []
    for W, a_s, a_d in zip(Ws, a_srcs, a_dsts):
        H, Fin, C = W.shape
        RW = _row_width(H, C)
        Wf = np.transpose(W, (1, 0, 2)).reshape(Fin, H * C)
        wsrc = np.einsum("hfc,hc->fh", W, a_s)
        wdst = np.einsum("hfc,hc->fh", W, a_d)
        Wa = np.zeros((Fin, RW), dtype=np.float32)
        Wa[:, : H * C] = Wf
        Wa[:, H * C : H * C + H] = wsrc
        Wa[:, H * C + H : H * C + 2 * H] = wdst
        W_augs.append(Wa.astype(BF16))
    return xTs, W_augs


# ----------------------------------------------------------------------------
# Device program
# ----------------------------------------------------------------------------

def build_nc(cfg, repeat=1):
    import concourse.bacc as bacc
    import concourse.mybir as mybir
    import concourse.tile as tile
    from contextlib import ExitStack

    f32 = mybir.dt.float32
    bf16 = mybir.dt.bfloat16
    i16 = mybir.dt.int16
    ALU = mybir.AluOpType
    ACT = mybir.ActivationFunctionType

    N = cfg["N"]
    NPAD = cfg["NPAD"]
    F_IN = cfg["F_IN"]
    C = cfg["C"]
    KB = cfg["KB"]
    HS = cfg["HS"]
    NPC = NPAD // NCORES
    B = NPC // 128
    ROWPAD = NPC + PADR
    NL = len(HS)
    RWs = [_row_width(H, C) for H in HS]
    FINs = [F_IN] + [C] * (NL - 1)
    SUMKB = sum(KB)
    off = [0]
    for kb in KB:
        off.append(off[-1] + kb * 128)

    nc = bacc.Bacc("TRN2", target_bir_lowering=False, debug=False,
                   num_devices=NCORES)

    # ---- I/O ----
    xT_d = nc.dram_tensor("xT", [F_IN, NPC], bf16, kind="ExternalInput")
    idx_d = nc.dram_tensor("idx", [128, SUMKB * 8], i16, kind="ExternalInput")
    W_d = [nc.dram_tensor(f"w{i+1}", [FINs[i], RWs[i]], bf16,
                          kind="ExternalInput") for i in range(NL)]
    bb_d = [nc.dram_tensor(f"bb{i+1}", [128, C], f32, kind="ExternalInput")
            for i in range(NL - 1)]
    b3r_d = nc.dram_tensor("b3r", [1, C], f32, kind="ExternalInput")
    out_d = nc.dram_tensor("out", [1, C], f32, kind="ExternalOutput")

    with tile.TileContext(nc, num_cores=NCORES) as tc, ExitStack() as ctx:
        dram = ctx.enter_context(tc.tile_pool(name="dram", bufs=1, space="DRAM"))
        cpool = ctx.enter_context(tc.tile_pool(name="consts", bufs=1))
        gpool = ctx.enter_context(tc.tile_pool(name="gath", bufs=1))
        wpool = ctx.enter_context(tc.tile_pool(name="work", bufs=1))
        psum = ctx.enter_context(tc.tile_pool(name="ps", bufs=1, space="PSUM"))

        hl = [dram.tile([ROWPAD, RWs[i]], bf16, tag=f"hl{i}", name=f"hl{i}")
              for i in range(NL)]
        hf = [dram.tile([ROWPAD * NCORES, RWs[i]], bf16, tag=f"hf{i}",
                        name=f"hf{i}") for i in range(NL)]

        # ---- constants into SBUF (outside the timed repeat loop) ----
        xT_sb = cpool.tile([F_IN, NPC], bf16, tag="xT")
        nc.sync.dma_start(xT_sb[:], xT_d[:, :])
        idx_sb = cpool.tile([128, SUMKB * 8], i16, tag="idx")
        nc.sync.dma_start(idx_sb[:], idx_d[:, :])
        W_sb = []
        for i in range(NL):
            w = cpool.tile([FINs[i], RWs[i]], bf16, tag=f"w{i}", name=f"w{i}")
            nc.sync.dma_start(w[:], W_d[i][:, :])
            W_sb.append(w)
        bb_sb = []
        for i in range(NL - 1):
            b = cpool.tile([128, C], f32, tag=f"bb{i}", name=f"bb{i}")
            nc.sync.dma_start(b[:], bb_d[i][:, :])
            bb_sb.append(b)
        b3_sb = cpool.tile([1, C], f32, tag="b3")
        nc.sync.dma_start(b3_sb[:], b3r_d[:, :])
        ones_sb = cpool.tile([128, 1], f32, tag="ones")
        nc.vector.memset(ones_sb[:], 1.0)
        # -inf pad rows (scores -> exp ~ 0; features multiplied by 0)
        ninf = cpool.tile([PADR, max(RWs)], bf16, tag="ninf")
        nc.vector.memset(ninf[:], -30000.0)
        for i in range(NL):
            nc.sync.dma_start(hl[i][NPC:ROWPAD, :], ninf[:, 0:RWs[i]])

        # next-layer transposed features
        x2T = [cpool.tile([128, NPC], bf16, tag=f"x2T{i}", name=f"x2T{i}")
               for i in range(NL - 1)]

        # hoisted num_idxs registers for the gathers (one per distinct value)
        kns = sorted({min(GC, kb - k0) for kb in KB for k0 in range(0, kb, GC)})
        kn_regs = {kn: nc.gpsimd.to_reg(kn * 128) for kn in kns}

        for _rep in range(repeat):
         for L in range(NL):
            H = HS[L]
            RW = RWs[L]
            FIN = FINs[L]
            HC = H * C
            HCF = min(HC, 512)   # feature matmul width (bank-limited)

            # ---- phase A: h_aug rows for own nodes ----
            staging = wpool.tile([128, B, RW], bf16, tag="stage")
            lhs = xT_sb if L == 0 else x2T[L - 1]

            # scores (transposed): scb[j, r] for j in [0, SCW)
            scb = wpool.tile([SCW, NPC], bf16, tag="scb")
            nchk = NPC // 512                     # 512-node score chunks
            for g0 in range(0, nchk, 3):
                gn = min(3, nchk - g0)
                psc = psum.tile([SCW, 3, 512], f32, tag="psc")
                for j in range(gn):
                    nc.tensor.matmul(
                        psc[:, j, :], W_sb[L][:, HC:HC + SCW],
                        lhs[:, (g0 + j) * 512:(g0 + j + 1) * 512],
                        start=True, stop=True)
                nc.scalar.copy(scb[:, g0 * 512:(g0 + gn) * 512],
                               psc[:, 0:gn, :])
            # scores -> staging[:, :, HC:HC+SCW] via XBAR transpose
            nc.sync.dma_start(staging[:, :, HC:HC + SCW], scb[:],
                              transpose=True)

            # features: 128-node chunks, groups of 3 PSUM banks
            for g0 in range(0, B, 3):
                gn = min(3, B - g0)
                pf = psum.tile([128, 3, 512], f32, tag="pf")
                for j in range(gn):
                    nc.tensor.matmul(
                        pf[:, j, 0:HCF], lhs[:, (g0 + j) * 128:(g0 + j + 1) * 128],
                        W_sb[L][:, 0:HCF], start=True, stop=True)
                nc.scalar.copy(staging[:, g0:g0 + gn, 0:HCF], pf[:, 0:gn, 0:HCF])
            if HC > HCF:
                for g0 in range(0, B, 3):
                    gn = min(3, B - g0)
                    pf = psum.tile([128, 3, 512], f32, tag="pf")
                    for j in range(gn):
                        nc.tensor.matmul(
                            pf[:, j, 0:HC - HCF],
                            lhs[:, (g0 + j) * 128:(g0 + j + 1) * 128],
                            W_sb[L][:, HCF:HC], start=True, stop=True)
                    nc.scalar.copy(staging[:, g0:g0 + gn, HCF:HC],
                                   pf[:, 0:gn, 0:HC - HCF])

            # staging -> hl rows r = chunk*128 + part
            nc.sync.dma_start(
                hl[L][0:NPC, :].rearrange("(ch p) w -> p ch w", p=128),
                staging[:])

            # ---- phase B: allgather ----
            nc.gpsimd.collective_compute(
                "AllGather", mybir.AluOpType.bypass,
                replica_groups=[list(range(NCORES))],
                ins=[hl[L][:, :].opt()],
                outs=[hf[L][:, :].opt()],
            )

            # s_dst for own nodes: block b slot s -> row s*B + b
            sdst = wpool.tile([128, B, H], bf16, tag="sdst")
            nc.sync.dma_start(
                sdst[:],
                hl[L][0:NPC, HC + H:HC + 2 * H].rearrange(
                    "(b s) h -> s b h", s=128))

            # accumulators over all blocks
            numer = wpool.tile([128, B, HC], f32, tag="numer")
            denom = wpool.tile([128, B, H], f32, tag="denom")

            # ---- phase C: per dst block ----
            for b in range(B):
                T = KB[b]
                g1 = gpool.tile([128, T, RW], bf16, tag="g1")
                for k0 in range(0, T, GC):
                    kn = min(GC, T - k0)
                    ic = slice((off[b] + k0 * 128) // 16,
                               (off[b] + (k0 + kn) * 128) // 16)
                    nc.gpsimd.dma_gather(g1[:, k0:k0 + kn, :], hf[L][:, :],
                                         idx_sb[:, ic], kn * 128, kn_regs[kn],
                                         RW, elem_step=RW)
                sc = wpool.tile([128, T, H], f32, tag="sc")
                nc.vector.tensor_tensor(
                    sc[:], g1[:, :, HC:HC + H],
                    sdst[:, b:b + 1, :].broadcast_to([128, T, H]), ALU.add)
                lr = wpool.tile([128, T, H], f32, tag="lr")
                nc.vector.scalar_tensor_tensor(lr[:], sc[:], 0.2, sc[:],
                                               ALU.mult, ALU.max)
                p = wpool.tile([128, T, H], f32, tag="p")
                nc.scalar.activation(p[:], lr[:], ACT.Exp)
                # msg = h_src * p, in place over gathered features
                gfeat = g1[:, :, 0:HC].rearrange("q t (h c) -> q t h c", h=H)
                nc.vector.tensor_tensor(
                    gfeat, gfeat,
                    p[:].unsqueeze(3).broadcast_to([128, T, H, C]), ALU.mult)
                nc.vector.tensor_reduce(
                    numer[:, b, :], g1[:, :, 0:HC].transpose([0, 2, 1]),
                    mybir.AxisListType.X, ALU.add)
                nc.vector.tensor_reduce(
                    denom[:, b, :], p[:].transpose([0, 2, 1]),
                    mybir.AxisListType.X, ALU.add)

            # ---- finalize (batched across blocks) ----
            nc.vector.tensor_scalar(denom[:], denom[:], 1e-16 * H, None,
                                    op0=ALU.add)
            rc = wpool.tile([128, B, H], f32, tag="rc")
            nc.vector.reciprocal(rc[:], denom[:])
            nview = numer[:].rearrange("q b (h c) -> q b h c", h=H)
            nc.vector.tensor_tensor(
                nview, nview,
                rc[:].unsqueeze(3).broadcast_to([128, B, H, C]), ALU.mult)
            if L < NL - 1:
                hm = wpool.tile([128, B, C], f32, tag="hm")
                nc.vector.tensor_reduce(
                    hm[:], nview.transpose([0, 1, 3, 2]),
                    mybir.AxisListType.X, ALU.add)
                m2 = wpool.tile([128, B, C], bf16, tag="m2")
                nc.vector.scalar_tensor_tensor(
                    m2[:], hm[:], 1.0 / H,
                    bb_sb[L][:].unsqueeze(1).broadcast_to([128, B, C]),
                    ALU.mult, ALU.add)
                nc.vector.tensor_scalar(m2[:], m2[:], 0.0, None, op0=ALU.max)
                nc.sync.dma_start(
                    x2T[L][:].rearrange("q (b s) -> q b s", b=B),
                    m2[:], transpose=True)
            else:
                nsum = wpool.tile([128, C], f32, tag="nsum")
                nc.vector.tensor_reduce(
                    nsum[:], numer[:].transpose([0, 2, 1]),
                    mybir.AxisListType.X, ALU.add)
                pfin = psum.tile([1, C], f32, tag="pfin")
                nc.tensor.matmul(pfin[:], ones_sb[:], nsum[:],
                                 start=True, stop=True)
                fs = wpool.tile([1, C], f32, tag="fs")
                nc.vector.scalar_tensor_tensor(fs[:], pfin[:], 1.0 / N,
                                               b3_sb[:], ALU.mult, ALU.add)
                nc.sync.dma_start(out_d[:, :], fs[:])

    nc.compile()
    return nc


# ----------------------------------------------------------------------------
# Entry points
# ----------------------------------------------------------------------------

def make_cfg_and_maps(inputs):
    x = np.asarray(inputs["x"])
    edge_index = np.asarray(inputs["edge_index"])
    N, F_IN = x.shape
    NPAD = ((N + 1023) // 1024) * 1024
    Ws = [np.asarray(inputs[f"W{i}"]) for i in (1, 2, 3)]
    a_srcs = [np.asarray(inputs[f"as{i}"]) for i in (1, 2, 3)]
    a_dsts = [np.asarray(inputs[f"ad{i}"]) for i in (1, 2, 3)]
    bs = [np.asarray(inputs[f"b{i}"]) for i in (1, 2, 3)]
    HS = tuple(W.shape[0] for W in Ws)
    C = Ws[0].shape[2]
    B = NPAD // NCORES // 128

    KB, idx_cores, perm = prep_static(edge_index, N, NPAD)
    xTs, W_augs = prep_values(x, Ws, a_srcs, a_dsts, perm, B)

    cfg = dict(N=N, NPAD=NPAD, F_IN=F_IN, C=C, KB=tuple(KB), HS=HS)
    in_maps = []
    for c in range(NCORES):
        m = {
            "xT": xTs[c],
            "idx": idx_cores[c],
            "b3r": (bs[2] * (1.0 / NCORES)).reshape(1, C).astype(np.float32),
        }
        for i in range(3):
            m[f"w{i+1}"] = W_augs[i]
        for i in range(2):
            m[f"bb{i+1}"] = np.broadcast_to(
                bs[i].astype(np.float32), (128, C)).copy()
        in_maps.append(m)
    return cfg, in_maps


_NC_CACHE = {}


def _get_nc(cfg, repeat=1):
    key = (repeat,) + tuple(sorted((k, v if not isinstance(v, tuple) else v)
                                   for k, v in cfg.items()))
    if key not in _NC_CACHE:
        _NC_CACHE[key] = build_nc(cfg, repeat=repeat)
    return _NC_CACHE[key]


def run(inputs, trace=False, repeat=1, **kw):
    from concourse.bass_utils import run_bass_kernel_spmd
    cfg, in_maps = make_cfg_and_maps(inputs)
    nc = _get_nc(cfg, repeat=repeat)
    res = run_bass_kernel_spmd(nc, in_maps, core_ids=list(range(NCORES)),
                               trace=trace, **kw)
    out = np.zeros((1, cfg["C"]), dtype=np.float32)
    for r in res.results:
        out += r["out"]
    return out, res


def kernel(**inputs) -> np.ndarray:
    out, _ = run(inputs)
    return out
